# revision 30
# baseline (speedup 1.0000x reference)
"""Paged GQA decode attention (B=64, HQ=32, HKV=8, D=128) on 8 TRN2 NeuronCores.

Strategy: data-parallel over requests with host-side load balancing + int8 KV.
 - Sort the 64 requests by context_lens descending; slot r of core c gets the
   rank-(r*8+c) request, so every core's slot-r request has a similar length.
 - Each slot is padded to the max-of-8 chunk count (chunks of 128 tokens), so
   all 8 cores execute the SAME static program (SPMD) on different data.
 - KV cache quantized to int8 on host (HBM traffic halved vs bf16):
     K: per-(kv-head, dim) scales, folded into q on host (q' = q*SCALE*sk) so
        the device never rescales scores.
     V: per-token scales sv; ln(sv) is folded into the exp bias so the ScalarE
        activation emits E' = exp(score+VSHIFT)*sv, and the PV matmul
        E'^T @ V_int8 == E^T @ V exactly (scales cancel per token). The
        softmax denominator uses a bf16 w=1/sv vector instead of ones.
 - K groups are cast-DMA'd int8->bf16 by SWDGE (integer-valued bf16 in SBUF);
   V groups land as int8 and are widened to bf16 by one VectorE copy per
   group (2 elem/cycle/lane), off the DMA critical path.
 - Per chunk on device: scores_T[l,hq] = K_h^T.T @ q'T (8 matmuls), then
   E' = exp(scores + bias) on ScalarE, then PV accumulation
   acc[hq,d] += E'_h.T @ V_h (8 col-tiled matmuls into two PSUM banks) and a
   w-matmul for the denominator. Final division happens on host.
"""

import math
import os
import sys
from contextlib import ExitStack

import numpy as np
import ml_dtypes  # noqa: F401  (numpy bf16 dtype)

for _p in ("/opt/trn_rl_repo", "/root/.axon_site/_ro/trn_rl_repo"):
    if os.path.isdir(_p) and _p not in sys.path:
        sys.path.insert(0, _p)
        break

import concourse.bass as bass  # noqa: F401
import concourse.tile as tile
from concourse import bacc, mybir
from concourse.bass_utils import run_bass_kernel_spmd

# bass_utils' trace path imports antenv.axon_hooks, which some images lack
# (the boot-time hook registration then degrades silently and NTFF profiling
# breaks). Synthesize the module and register the ctypes hook if missing.
try:
    from antenv import axon_hooks as _axon_hooks  # noqa: F401
except ImportError:
    import types as _types

    import antenv as _antenv

    _m = _types.ModuleType("antenv.axon_hooks")
    _m._hook = None

    def _set_hook(h, _m=_m):
        _m._hook = h

    def _get_hook(_m=_m):
        return _m._hook

    _m.set_axon_ntff_profile_hook = _set_hook
    _m.get_axon_ntff_profile_hook = _get_hook
    sys.modules["antenv.axon_hooks"] = _m
    _antenv.axon_hooks = _m
    try:
        from trn_agent_boot.trn_boot import _ntff_profile_via_ctypes

        _m._hook = _ntff_profile_via_ctypes("/opt/axon/libaxon_pjrt.so")
    except Exception:
        pass

B, HQ, HKV, D, BS, MB = 64, 32, 8, 128, 16, 128
G = HQ // HKV              # 4 query heads per kv head
SCALE = 0.08838834764831845
NCORES = 8
SLOTS = B // NCORES        # 8 request slots per core
CHUNK = 128                # tokens per chunk (= SBUF partitions)
BPC = CHUNK // BS          # blocks per chunk = 8
ROW = HKV * D              # 1024 elements per token row
NEG = -30.0                # additive mask for invalid tokens
VSHIFT = -2.0              # shift scores so exp() stays well-conditioned
GRP = 8                    # chunks per DMA group (groups may span slots)
K_BUFS = 5                 # K group tiles in flight (bf16, 16KB/partition)
V_BUFS = 5                 # V int8 group tiles in flight (8KB/partition)
VB_BUFS = 4                # bf16 dequant tiles in flight (16KB/partition)
K_DVE_GROUPS = {0, 3}      # K groups widened on VectorE instead of cast-DMA
SYNC_GROUPS = {0}          # groups DMA'd via HWDGE: skips the ~7us SWDGE
                           # ring-init preamble so first compute starts early

BF16 = ml_dtypes.bfloat16

last_results = None        # stashed BassKernelResults for test.py

_prog_cache = {}


def _group_sizes(C_total):
    """Ramped DMA group sizes: small head/tail groups shorten the fill and
    drain latency; GRP-sized groups in the body for DMA efficiency."""
    if C_total <= GRP:
        return [C_total]
    head = tail = GRP // 2
    body = C_total - head - tail
    sizes = [head] + [GRP] * (body // GRP)
    if body % GRP:
        sizes.append(body % GRP)
    sizes.append(tail)
    return sizes


def _chunk_map(sizes):
    m = []
    for g, sz in enumerate(sizes):
        for h in range(sz):
            m.append((g, h))
    return m


def _build_program(s_counts):
    f32 = mybir.dt.float32
    bf16 = mybir.dt.bfloat16
    i8 = mybir.dt.int8
    C_total = sum(s_counts)
    sizes = _group_sizes(C_total)
    goff = [0]
    for sz in sizes:
        goff.append(goff[-1] + sz)
    cmap = _chunk_map(sizes)
    nc = bacc.Bacc()

    k_d = nc.declare_dram_parameter("k", [D, C_total * ROW], i8,
                                    isOutput=False)
    v_d = nc.declare_dram_parameter("v", [CHUNK, C_total * ROW], i8,
                                    isOutput=False)
    qT_d = nc.declare_dram_parameter("qT", [D, SLOTS * HQ], bf16,
                                     isOutput=False)
    bias_d = nc.declare_dram_parameter("bias", [CHUNK, C_total], f32,
                                       isOutput=False)
    w_d = nc.declare_dram_parameter("w", [CHUNK, C_total], bf16,
                                    isOutput=False)
    # packed outputs: one DMA each at stream end (tiny per-slot DMAs would
    # serialize ~600ns apiece on the sync queue and form a long tail).
    outa_d = nc.declare_dram_parameter("outa", [CHUNK, SLOTS * D], f32,
                                       isOutput=True)
    outb_d = nc.declare_dram_parameter("outb", [CHUNK, SLOTS * D], f32,
                                       isOutput=True)
    den_d = nc.declare_dram_parameter("den", [HQ, SLOTS], f32, isOutput=True)

    EXP = mybir.ActivationFunctionType.Exp

    with tile.TileContext(nc) as tc, ExitStack() as ctx:
        kpool = ctx.enter_context(tc.tile_pool(name="kp", bufs=K_BUFS))
        vpool = ctx.enter_context(tc.tile_pool(name="vp", bufs=V_BUFS))
        vbpool = ctx.enter_context(tc.tile_pool(name="vb", bufs=VB_BUFS))
        epool = ctx.enter_context(tc.tile_pool(name="e", bufs=4))
        const = ctx.enter_context(tc.tile_pool(name="cst", bufs=1))
        spsum = ctx.enter_context(tc.tile_pool(name="sp", bufs=3, space="PSUM"))
        apsum = ctx.enter_context(tc.tile_pool(name="ac", bufs=2, space="PSUM"))
        dpsum = ctx.enter_context(tc.tile_pool(name="dp", bufs=2, space="PSUM"))

        # consts ride the scalar HWDGE ring so the sync ring starts on the
        # first V group immediately.
        bias_t = const.tile([CHUNK, C_total], f32)
        nc.scalar.dma_start(bias_t[:], bias_d[:])
        w_t = const.tile([CHUNK, C_total], bf16)
        nc.scalar.dma_start(w_t[:], w_d[:])
        q_all = const.tile([D, SLOTS * HQ], bf16)
        nc.scalar.dma_start(q_all[:], qT_d[:])
        # dummy matmuls absorb the const-DMA waits so real matmuls only wait
        # on their K/V tiles.
        dmy = spsum.tile([1, 1], f32, tag="sco")
        nc.tensor.matmul(dmy[:], q_all[0:1, 0:1], q_all[0:1, 0:1],
                         start=True, stop=True)
        dmy2 = spsum.tile([1, 1], f32, tag="sco")
        nc.tensor.matmul(dmy2[:], w_t[0:1, 0:1], w_t[0:1, 0:1],
                         start=True, stop=True)

        outa_all = const.tile([CHUNK, SLOTS * D], f32)
        outb_all = const.tile([CHUNK, SLOTS * D], f32)
        den_all = const.tile([HQ, SLOTS], f32)

        cur = {}

        def load_group(g):
            gsz = sizes[g]
            o0, o1 = goff[g] * ROW, goff[g + 1] * ROW
            ksrc = k_d[:, o0:o1]
            vsrc = v_d[:, o0:o1]
            hw = gsz * ROW // 2
            eng = nc.sync if g in SYNC_GROUPS else nc.gpsimd
            if g in K_DVE_GROUPS:
                ki = kpool.tile([D, gsz * ROW], i8, tag="kg", name="kg")
                eng.dma_start(ki[:], ksrc)
                kb = vbpool.tile([D, gsz * ROW], bf16, tag="vb", name="kb")
                nc.vector.tensor_copy(kb[:, :hw], ki[:, :hw])
                nc.vector.tensor_copy(kb[:, hw:], ki[:, hw:])
                cur["k"] = kb
            else:
                kb = kpool.tile([D, gsz * ROW], bf16, tag="kg", name="kg")
                nc.gpsimd.dma_start(kb[:], ksrc)
                cur["k"] = kb
            vi = vpool.tile([CHUNK, gsz * ROW], i8, tag="vg", name="vg")
            eng.dma_start(vi[:], vsrc)
            vb = vbpool.tile([CHUNK, gsz * ROW], bf16, tag="vb", name="vb")
            nc.vector.tensor_copy(vb[:, :hw], vi[:, :hw])
            nc.vector.tensor_copy(vb[:, hw:], vi[:, hw:])
            cur["v"] = vb

        gc = 0
        for r in range(SLOTS):
            S_r = s_counts[r]
            qt = q_all[:, r * HQ:(r + 1) * HQ]
            acc = apsum.tile([CHUNK, 2 * D], f32, tag="acc")
            acc_a, acc_b = acc[:, 0:D], acc[:, D:2 * D]
            den_p = dpsum.tile([HQ, 1], f32, tag="den")
            for j in range(S_r):
                g, half = cmap[gc + j]
                if half == 0 or "k" not in cur:
                    load_group(g)
                kt = cur["k"][:, half * ROW:(half + 1) * ROW]
                vt = cur["v"][:, half * ROW:(half + 1) * ROW]

                sco = spsum.tile([CHUNK, HQ], f32, tag="sco")
                for h in range(HKV):
                    nc.tensor.matmul(
                        sco[:, h * G:(h + 1) * G],
                        kt[:, h * D:(h + 1) * D],
                        qt[:, h * G:(h + 1) * G],
                        start=True, stop=True,
                    )
                et = epool.tile([CHUNK, HQ], bf16)
                nc.scalar.activation(
                    et[:], sco[:], EXP,
                    bias=bias_t[:, gc + j:gc + j + 1], scale=1.0,
                )
                st, sp = (j == 0), (j == S_r - 1)
                for h in range(HKV):
                    accp = acc_a if h < 4 else acc_b
                    jj = h % 4
                    # acc_a/acc_b share PSUM bank rows; start=True clears the
                    # whole 2KiB row, so only the acc_a half may assert it.
                    # acc_b's first write lands on cleared has_written bits
                    # and overwrites (flags=0 semantics).
                    nc.tensor.matmul(
                        accp[32 * jj:32 * jj + G, :],
                        et[:, h * G:(h + 1) * G],
                        vt[:, h * D:(h + 1) * D],
                        start=st and h < 4, stop=sp,
                        tile_position=(0, 32 * jj),
                        skip_group_check=h >= 4,
                    )
                nc.tensor.matmul(den_p[:], et[:],
                                 w_t[:, gc + j:gc + j + 1],
                                 start=st, stop=sp)
            nc.scalar.copy(outa_all[:, r * D:(r + 1) * D], acc_a[:])
            nc.scalar.copy(outb_all[:, r * D:(r + 1) * D], acc_b[:])
            nc.scalar.copy(den_all[:, r:r + 1], den_p[:])
            gc += S_r
        nc.sync.dma_start(outa_d[:], outa_all[:])
        nc.sync.dma_start(outb_d[:], outb_all[:])
        nc.sync.dma_start(den_d[:], den_all[:])
    nc.compile()
    return nc


def _get_program(s_counts):
    if s_counts not in _prog_cache:
        _prog_cache[s_counts] = _build_program(s_counts)
    return _prog_cache[s_counts]


def _make_schedule(context_lens):
    L = context_lens.astype(np.int64)
    order = np.argsort(-L, kind="stable")
    s_counts = []
    for r in range(SLOTS):
        grp = order[r * NCORES:(r + 1) * NCORES]
        s_counts.append(max(1, math.ceil(int(L[grp].max()) / CHUNK)))
    rem = (-sum(s_counts)) % 2
    s_counts[-1] += rem  # pad stream to an even chunk count (tail group)
    return order, tuple(s_counts)


def _quantize_caches(k_cache, v_cache):
    """int8-quantize the caches once (shared across cores).

    K: per-(kv-head, dim) scales sk[HKV, D] (folded into q later).
    V: per-token scales sv[nblocks, BS] over each token's HKV*D row.
    """
    nb = k_cache.shape[0]
    sk = np.abs(k_cache).max(axis=(0, 1)) / 127.0          # [HKV, D]
    sk = np.maximum(sk, 1e-12).astype(np.float32)
    kq = np.clip(np.round(k_cache / sk[None, None]), -127, 127) \
        .astype(np.int8)
    vflat = v_cache.reshape(nb, BS, ROW)
    sv = np.abs(vflat).max(axis=2) / 127.0                 # [nb, BS]
    sv = np.maximum(sv, 1e-12).astype(np.float32)
    vq = np.clip(np.round(vflat / sv[:, :, None]), -127, 127) \
        .astype(np.int8)
    return kq.reshape(nb, BS, ROW), vq, sk, sv


def _build_in_maps(q, kq, vq, sk, sv, block_tables, L, order, s_counts):
    C_total = sum(s_counts)
    nblocks_total = kq.shape[0]

    in_maps = []
    core_reqs = []
    for c in range(NCORES):
        karr = np.empty((C_total, D, ROW), np.int8)
        varr = np.empty((C_total, CHUNK, ROW), np.int8)
        biasT = np.empty((C_total, CHUNK), np.float32)
        wT = np.empty((C_total, CHUNK), np.float32)
        qT = np.empty((D, SLOTS * HQ), BF16)
        reqs = []
        gc = 0
        for r in range(SLOTS):
            b = int(order[r * NCORES + c])
            reqs.append(b)
            S_r = s_counts[r]
            blocks = np.clip(block_tables[b, :S_r * BPC].astype(np.int64),
                             0, nblocks_total - 1)
            kreq = kq[blocks].reshape(S_r, CHUNK, HKV, D)
            karr[gc:gc + S_r] = \
                kreq.transpose(0, 3, 2, 1).reshape(S_r, D, ROW)
            varr[gc:gc + S_r] = vq[blocks].reshape(S_r, CHUNK, ROW)
            svtok = sv[blocks].reshape(S_r * CHUNK)
            tok = np.arange(S_r * CHUNK, dtype=np.int64)
            valid = tok < int(L[b])
            biasT[gc:gc + S_r] = np.where(
                valid, VSHIFT + np.log(svtok), NEG) \
                .astype(np.float32).reshape(S_r, CHUNK)
            wT[gc:gc + S_r] = np.where(valid, 1.0 / svtok, 0.0) \
                .reshape(S_r, CHUNK)
            # fold K scales into q: q'[d, h] = q[h, d]*SCALE*sk[h//G, d]
            qs = (q[b] * SCALE).reshape(HKV, G, D) * sk[:, None, :]
            qT[:, r * HQ:(r + 1) * HQ] = \
                qs.reshape(HQ, D).T.astype(BF16)
            gc += S_r
        # flat chunk-major layout: group g = column slice of k/v
        kflat = np.ascontiguousarray(
            karr.transpose(1, 0, 2)).reshape(D, C_total * ROW)
        vflat = np.ascontiguousarray(
            varr.transpose(1, 0, 2)).reshape(CHUNK, C_total * ROW)
        in_maps.append({
            "k": kflat, "v": vflat, "qT": qT,
            "bias": np.ascontiguousarray(biasT.T),
            "w": np.ascontiguousarray(wT.T).astype(BF16),
        })
        core_reqs.append(reqs)
    return in_maps, core_reqs


def kernel(q, k_cache, v_cache, block_tables, context_lens):
    global last_results
    q = np.asarray(q, dtype=np.float32)
    k_cache = np.asarray(k_cache, dtype=np.float32)
    v_cache = np.asarray(v_cache, dtype=np.float32)
    block_tables = np.asarray(block_tables, dtype=np.int32)
    context_lens = np.asarray(context_lens, dtype=np.int32)

    L = context_lens.astype(np.int64)
    order, s_counts = _make_schedule(context_lens)
    nc = _get_program(s_counts)
    kq, vq, sk, sv = _quantize_caches(k_cache, v_cache)
    in_maps, core_reqs = _build_in_maps(
        q, kq, vq, sk, sv, block_tables, L, order, s_counts)

    res = run_bass_kernel_spmd(
        nc, in_maps, list(range(NCORES)),
        trace=bool(os.environ.get("KBASS_TRACE")),
    )
    last_results = res

    out = np.empty((B, HQ, D), np.float32)
    for c in range(NCORES):
        # outa/outb: [128, SLOTS*D]; head h (<4 in a, >=4 in b) sub-head g
        # lives on partition 32*h' + g where h' = h % 4.
        oa = res.results[c]["outa"].reshape(4, 32, SLOTS, D)
        ob = res.results[c]["outb"].reshape(4, 32, SLOTS, D)
        acc = np.concatenate([oa[:, :G], ob[:, :G]], axis=0) \
            .transpose(2, 0, 1, 3).reshape(SLOTS, HQ, D)
        den = np.maximum(res.results[c]["den"], 1e-30).T  # [SLOTS, HQ]
        o = acc / den[:, :, None]
        for r, b in enumerate(core_reqs[c]):
            out[b] = o[r]
    return out
